# revision 1
# baseline (speedup 1.0000x reference)
"""DC_CE_Marginal_loss for Trainium2 — 8-core data-parallel Bass kernel.

Shards the [B,C,D,H,W] volume along D across 8 NeuronCores, two launches:

  Launch A: each core loads its target shard (bf16; one-hot is exact) and
      computes local per-(b,c) voxel counts (free-dim reductions split over
      ScalarE and VectorE). Host sums the 8x[128,16] partials into global
      counts — the "psum of present-class counts" — and derives the
      presence masks / merge weights / CE padding (40 floats).

  Launch B: each core streams its net_output shard and computes per chunk:
      merged background logit (masked scalar_tensor_tensor chain), masked
      exp (ACT, additive -1e9 bias), softmax denominator S (pairwise adds),
      fast reciprocal, then fused affine_mul_reduce ops that produce
      softmax q while accumulating seg_vol / intersect / sum(t*m) into
      per-chunk columns; ACT Log accumulates sum(log(S+pad)).

Host sums the per-core/per-chunk partial columns and finishes the loss.
"""
import numpy as np
import ml_dtypes

B, C, D, H, W = 2, 8, 64, 160, 160
NCORES = 8
DS = D // NCORES            # depth slices per core
PLANE = DS * H * W          # voxels per (b,c) plane per core = 204800
P = 128
FREE = PLANE // P           # 1600
NCH = 4                     # chunks per sample plane
FCH = FREE // NCH           # 400
BIG = 1e9
NVOX = B * D * H * W

# launch B per-chunk accumulator columns: base = (b*NCH+ch)*CPC
CPC = 25          # seg[0:8], intersect[8:16], u-terms[16:24], lse[24]
NOUT = B * NCH * CPC
# masks input columns
MK_BM = 0         # 16: additive exp mask (0 present / -BIG absent)
MK_A = 16         # 16: 1-present (merge weights)
MK_PAD = 32       # 2: CE padding per sample
NMASK = 40

_CACHE = {}


def _build_a():
    import concourse.bacc as bacc
    import concourse.tile as tile
    from concourse import mybir

    FA = mybir.ActivationFunctionType
    AL = mybir.AluOpType
    f32, bf16 = mybir.dt.float32, mybir.dt.bfloat16

    nc = bacc.Bacc("TRN2", num_devices=NCORES, name="loss_counts")
    t = nc.dram_tensor("t", [B * C, P, FREE], bf16, kind="ExternalInput")
    out = nc.dram_tensor("cnt", [P, B * C], f32, kind="ExternalOutput")

    with tile.TileContext(nc) as tc:
        with (
            tc.tile_pool(name="tin", bufs=4) as tin,
            tc.tile_pool(name="sb", bufs=1) as sb,
        ):
            cnt = sb.tile([P, B * C], f32)
            junk_a = sb.tile([P, 2 * FREE], bf16)
            for g in range(8):  # 2 planes per DMA; reduce on DVE or ACT
                t_sb = tin.tile([P, 2, FREE], bf16, tag="t")
                src = t[2 * g : 2 * g + 2, :, :].rearrange("c p f -> p c f")
                nc.sync.dma_start(t_sb[:], src)
                if g % 2 == 0:
                    nc.vector.tensor_reduce(
                        out=cnt[:, 2 * g : 2 * g + 2], in_=t_sb[:],
                        axis=mybir.AxisListType.X, op=AL.add)
                else:
                    for j in range(2):
                        nc.scalar.activation(
                            out=junk_a[:, j * FREE : (j + 1) * FREE],
                            in_=t_sb[:, j, :], func=FA.Copy,
                            accum_out=cnt[:, 2 * g + j : 2 * g + j + 1])
            nc.sync.dma_start(out[:], cnt[:])
    nc.compile()
    return nc


import os
BG_GPSIMD = os.environ.get("K_BG_GPSIMD", "0") == "1"
E_BF16 = os.environ.get("K_E_BF16", "1") == "1"
I_ON_ACT = os.environ.get("K_I_ON_ACT", "1") == "1"
UM_GPSIMD = os.environ.get("K_UM_GPSIMD", "0") == "1"
UM_WIDE_TT = os.environ.get("K_UM_WIDE_TT", "1") == "1"


def _build_b():
    import concourse.bacc as bacc
    import concourse.tile as tile
    from concourse import mybir

    FA = mybir.ActivationFunctionType
    AL = mybir.AluOpType
    f32, bf16 = mybir.dt.float32, mybir.dt.bfloat16
    edt = bf16 if E_BF16 else f32

    nc = bacc.Bacc("TRN2", num_devices=NCORES, name="loss_main")
    x = nc.dram_tensor("x", [B * C, P, FREE], f32, kind="ExternalInput")
    t = nc.dram_tensor("t", [B * C, P, FREE], bf16, kind="ExternalInput")
    masks = nc.dram_tensor("masks", [P, NMASK], f32, kind="ExternalInput")
    out = nc.dram_tensor("out", [P, NOUT], f32, kind="ExternalOutput")

    beng = nc.gpsimd if BG_GPSIMD else nc.vector

    with tile.TileContext(nc) as tc:
        with (
            tc.tile_pool(name="persist", bufs=1) as persist,
            tc.tile_pool(name="xin", bufs=3) as xin,
            tc.tile_pool(name="ework", bufs=2) as ework,
            tc.tile_pool(name="qwork", bufs=2) as qwork,
            tc.tile_pool(name="swork", bufs=2) as swork,
        ):
            mk = persist.tile([P, NMASK], f32)
            nc.sync.dma_start(mk[:], masks[:])
            # prefetch chunk-0 logits before the (large) target loads so the
            # first chunk's DVE work isn't gated on all 6.6MB of t
            x_ch0 = xin.tile([P, C, FCH], f32, tag="x", name="x_ch0")
            nc.sync.dma_start(
                x_ch0[:], x[0:C, :, 0:FCH].rearrange("c p f -> p c f"))
            t_sb = persist.tile([P, B * C, FREE], bf16)
            for bc in range(B * C):
                nc.sync.dma_start(t_sb[:, bc, :], t[bc])
            accs = persist.tile([P, NOUT], f32)
            nc.vector.memset(accs[:], 0.0)
            junk_dve = persist.tile([P, C, FCH], f32)
            # all S chunks retained so the Ln ops run back-to-back at the
            # end (one act-table load instead of per-chunk exp<->ln flips)
            S_all = persist.tile([P, B * NCH, FCH], f32)

            for b in range(B):
                for ch in range(NCH):
                    sl = slice(ch * FCH, (ch + 1) * FCH)
                    base = (b * NCH + ch) * CPC
                    if b == 0 and ch == 0:
                        x_ch = x_ch0
                    else:
                        x_ch = xin.tile([P, C, FCH], f32, tag="x")
                        src = x[b * C : (b + 1) * C, :, sl].rearrange(
                            "c p f -> p c f")
                        nc.sync.dma_start(x_ch[:], src)

                    # bg = sum_{c>=1} absent_c * x_c ; x_0 += bg (merged logit)
                    bg = swork.tile([P, FCH], f32, tag="bg")
                    beng.tensor_scalar(
                        bg[:], x_ch[:, 1, :],
                        mk[:, MK_A + b * C + 1 : MK_A + b * C + 2], None, AL.mult)
                    for c in range(2, C):
                        bg2 = swork.tile([P, FCH], f32, tag="bg")
                        beng.scalar_tensor_tensor(
                            out=bg2[:], in0=x_ch[:, c, :],
                            scalar=mk[:, MK_A + b * C + c : MK_A + b * C + c + 1],
                            in1=bg[:], op0=AL.mult, op1=AL.add)
                        bg = bg2
                    beng.scalar_tensor_tensor(
                        out=x_ch[:, 0, :], in0=x_ch[:, 0, :], scalar=1.0,
                        in1=bg[:], op0=AL.mult, op1=AL.add)

                    # e_c = exp(m_c + mask_bias_c)
                    e_ch = ework.tile([P, C, FCH], edt, tag="e")
                    for c in range(C):
                        last_exp = nc.scalar.activation(
                            out=e_ch[:, c, :], in_=x_ch[:, c, :],
                            func=FA.Exp,
                            bias=mk[:, MK_BM + b * C + c : MK_BM + b * C + c + 1],
                            scale=1.0)

                    # S = sum_c e_c (pairwise tree on wide slices)
                    s4 = swork.tile([P, 4, FCH], edt, tag="s4")
                    nc.vector.tensor_tensor(out=s4[:], in0=e_ch[:, 0:4, :],
                                            in1=e_ch[:, 4:8, :], op=AL.add)
                    s2 = swork.tile([P, 2, FCH], edt, tag="s2")
                    nc.vector.tensor_tensor(out=s2[:], in0=s4[:, 0:2, :],
                                            in1=s4[:, 2:4, :], op=AL.add)
                    S = S_all[:, b * NCH + ch, :]
                    nc.vector.tensor_tensor(out=S, in0=s2[:, 0, :],
                                            in1=s2[:, 1, :], op=AL.add)

                    r = swork.tile([P, FCH], f32, tag="r")
                    nc.vector.reciprocal_approx_fast(r[:], S)

                    # q_c = e_c * r ; seg_c = sum(q_c)  (fused custom DVE op)
                    q_ch = qwork.tile([P, C, FCH], edt, tag="q")
                    for c in range(C):
                        nc.vector.affine_mul_reduce(
                            out=q_ch[:, c, :],
                            accum_out=accs[:, base + c : base + c + 1],
                            in0=e_ch[:, c, :], in1=r[:], scale=1.0, bias=0.0)
                    # intersect_c = sum(t_c * q_c)
                    if I_ON_ACT:
                        tq_ch = qwork.tile([P, C, FCH], edt, tag="tq")
                        nc.vector.tensor_tensor(
                            out=tq_ch[:], in0=t_sb[:, b * C : (b + 1) * C, sl],
                            in1=q_ch[:], op=AL.mult)
                        for c in range(C):
                            nc.scalar.activation(
                                out=tq_ch[:, c, :], in_=tq_ch[:, c, :],
                                func=FA.Copy,
                                accum_out=accs[:, base + 8 + c : base + 9 + c])
                    else:
                        for c in range(C):
                            nc.vector.affine_mul_reduce(
                                out=junk_dve[:, 0, :],
                                accum_out=accs[:, base + 8 + c : base + 9 + c],
                                in0=t_sb[:, b * C + c, sl], in1=q_ch[:, c, :],
                                scale=1.0, bias=0.0)
                    # u-term = sum_c sum(t_c * m_c)   (x_0 already merged)
                    if UM_WIDE_TT:
                        um_ch = qwork.tile([P, C, FCH], f32, tag="um")
                        ueng = nc.gpsimd if UM_GPSIMD else nc.vector
                        ueng.tensor_tensor(
                            out=um_ch[:], in0=t_sb[:, b * C : (b + 1) * C, sl],
                            in1=x_ch[:, :, :], op=AL.mult)
                        nc.scalar.activation(
                            out=um_ch[:], in_=um_ch[:], func=FA.Copy,
                            accum_out=accs[:, base + 16 : base + 17])
                    else:
                        for c in range(C):
                            nc.vector.affine_mul_reduce(
                                out=junk_dve[:, 0, :],
                                accum_out=accs[:, base + 16 + c : base + 17 + c],
                                in0=t_sb[:, b * C + c, sl],
                                in1=x_ch[:, c, :],
                                scale=1.0, bias=0.0)

            # CE lse terms at the end: sum(log(S + pad_b)) via ACT accum
            junk_act = persist.tile([P, FCH], f32)
            from concourse.tile import add_dep_helper
            for b in range(B):
                for ch in range(NCH):
                    base = (b * NCH + ch) * CPC
                    ln_inst = nc.scalar.activation(
                        out=junk_act[:], in_=S_all[:, b * NCH + ch, :],
                        func=FA.Ln,
                        bias=mk[:, MK_PAD + b : MK_PAD + b + 1], scale=1.0,
                        accum_out=accs[:, base + 24 : base + 25])
                    # keep every Ln after the final Exp so the activation
                    # table set is switched exactly once
                    add_dep_helper(ln_inst.ins, last_exp.ins, False,
                                   "batch ln after exps")

            nc.sync.dma_start(out[:], accs[:])
    nc.compile()
    return nc


def _get(name, builder):
    if name not in _CACHE:
        _CACHE[name] = builder()
    return _CACHE[name]


def _shard_inputs(net_output, target):
    xs = np.ascontiguousarray(net_output).reshape(B, C, NCORES, P, FREE)
    ts = np.ascontiguousarray(target).reshape(B, C, NCORES, P, FREE)
    xmaps, tmaps = [], []
    for k in range(NCORES):
        xk = np.ascontiguousarray(xs[:, :, k]).reshape(B * C, P, FREE)
        tk = np.ascontiguousarray(ts[:, :, k]).reshape(B * C, P, FREE)
        xmaps.append(xk)
        tmaps.append(tk.astype(ml_dtypes.bfloat16))  # one-hot: exact in bf16
    return xmaps, tmaps


def _masks_from_counts(cnt_g):
    """cnt_g [B,C] -> (masks [P,NMASK] f32, present, n)"""
    present = cnt_g > 0.5
    pm = present.astype(np.float32)
    n = pm.sum(axis=1)
    L = n.max()
    pad = (L - n).astype(np.float32)
    mrow = np.zeros((NMASK,), dtype=np.float32)
    mrow[MK_BM : MK_BM + B * C] = pm.reshape(-1) * BIG - BIG
    mrow[MK_A : MK_A + B * C] = 1.0 - pm.reshape(-1)
    mrow[MK_PAD : MK_PAD + B] = pad
    masks = np.ascontiguousarray(np.broadcast_to(mrow, (P, NMASK)))
    return masks, present, n


def _run(nc, in_maps, out_name):
    if os.environ.get("K_SIM", "0") == "1":
        import concourse.bass_interp as bass_interp
        sim = bass_interp.MultiCoreSim(nc, NCORES)
        for k in range(NCORES):
            for name, arr in in_maps[k].items():
                sim.cores[k].tensor(name)[:] = arr
        sim.simulate()
        return [{out_name: sim.cores[k].tensor(out_name).copy()}
                for k in range(NCORES)]
    from concourse.bass_utils import run_bass_kernel_spmd
    return run_bass_kernel_spmd(
        nc, in_maps, core_ids=list(range(NCORES))).results


def run_a(tmaps):
    nc = _get("a", _build_a)
    results = _run(nc, [{"t": tk} for tk in tmaps], "cnt")
    cnt_g = np.zeros((B, C), dtype=np.float64)
    for r in results:
        cnt_g += r["cnt"].astype(np.float64).sum(axis=0).reshape(B, C)
    return cnt_g


def run_b(xmaps, tmaps, masks):
    nc = _get("b", _build_b)
    in_maps = [{"x": xmaps[k], "t": tmaps[k], "masks": masks}
               for k in range(NCORES)]
    results = _run(nc, in_maps, "out")
    acc = np.zeros((NOUT,), dtype=np.float64)
    for r in results:
        acc += r["out"].astype(np.float64).sum(axis=0)
    return acc


def _finish(cnt_g, acc, present, n):
    cols = acc.reshape(B, NCH, CPC).sum(axis=1)   # [B, CPC]
    seg = cols[:, 0:8]
    inter = cols[:, 8:16]
    u = cols[:, 16:24].sum(axis=1)                # [B]
    lse_sum = cols[:, 24]
    ce = (lse_sum.sum() - u.sum()) / NVOX
    dice_c = 2.0 * inter / (cnt_g + seg + 1e-5)
    dice_i = 1.0 - (present * dice_c).sum(axis=1) / n
    dc = dice_i.mean()
    return np.asarray(0.5 * ce + 0.5 * dc, dtype=np.float32)


def kernel(net_output, target):
    xmaps, tmaps = _shard_inputs(np.asarray(net_output), np.asarray(target))
    cnt_g = run_a(tmaps)
    masks, present, n = _masks_from_counts(cnt_g)
    acc = run_b(xmaps, tmaps, masks)
    return _finish(cnt_g, acc, present, n)



# revision 7
# speedup vs baseline: 2.3238x; 2.3238x over previous
"""DC_CE_Marginal_loss for Trainium2 — 8-core data-parallel Bass kernel.

Single fused NEFF per core (D-sharded, pure data parallel). The program is
JIT-specialized on the per-sample present-class pattern (scanned on host at
build time); the device re-derives the label counts every run (PE matmul
pass over the one-hot target) and the host verifies the pattern against the
baked one, rebuilding + rerunning on mismatch — so any input is handled
correctly, and the common path is a single compile per process.

Math (masked re-expression, per sample b with present set Pb, absent set
Ab, pad_b = max |Pb'| - |Pb|):
  m_0   = x_0 + sum_{a in Ab} x_a        (merge_prediction)
  e_c   = exp(m_c) for c in Pb;  S = sum_c e_c;  q_c = e_c / S
  seg_c = sum_v q_c;  inter_c = sum_v t_c q_c;  ql = sum_c t_c q_c
  CE    = mean_v [ln(S+pad) - ln S - ln ql]  (pad=0 -> just -mean ln ql)
  dice from seg/inter/counts as usual (host, tiny).

Engine mapping per (sample, chunk): DVE runs the bf16 trees/products in 2x
mode; ACT runs exp/ln/copy; PE (otherwise idle) does every per-class
reduction as ones-stationary matmuls folding into PSUM (counts, seg,
inter); the host finishes the tiny cross-core scalar reduction.
"""
import os
import numpy as np
import ml_dtypes

B, C, D, H, W = 2, 8, 64, 160, 160
NCORES = 8
DS = D // NCORES
PLANE = DS * H * W          # voxels per (b,c) plane per core = 204800
P = 128
FREE = PLANE // P           # 1600
FCH = 800                   # chunk free size
NCH = FREE // FCH           # chunks per sample
NVOX = B * D * H * W

CNT_FOLD = 32               # counts psum fold width
ACC_FOLD = 50               # seg/inter psum fold width (FCH = 16*50)

K_SIM = os.environ.get("K_SIM", "0") == "1"
K_BCAST = os.environ.get("K_BCAST", "1") == "1"
K_DEFER_LN = os.environ.get("K_DEFER_LN", "1") == "1"

_CACHE: dict = {}


def _spec_from_presence(pres):
    """pres: [B, C] bool -> per-sample present/absent lists and pads."""
    pl = [[c for c in range(C) if pres[b, c]] for b in range(B)]
    al = [[c for c in range(C) if not pres[b, c]] for b in range(B)]
    n = [len(p) for p in pl]
    mx = max(n)
    pad = [float(mx - nb) for nb in n]
    assert all(0 in p for p in pl), "background must be present"
    return pl, al, pad


def _build(pres_key):
    import concourse.bacc as bacc
    import concourse.tile as tile
    from concourse import mybir
    from concourse.tile import add_dep_helper
    from concourse.bass import broadcast_tensor_aps

    pres = np.array(pres_key, dtype=bool).reshape(B, C)
    PL, ABS, PAD = _spec_from_presence(pres)

    FA = mybir.ActivationFunctionType
    OP = mybir.AluOpType
    f32, bf16 = mybir.dt.float32, mybir.dt.bfloat16

    nc = bacc.Bacc("TRN2", num_devices=NCORES, name="loss_fused")
    x = nc.dram_tensor("x", [B * C, P, FREE], bf16, kind="ExternalInput")
    t = nc.dram_tensor("t", [B * C, P, FREE], bf16, kind="ExternalInput")
    nlnc = sum((3 if PAD[b] > 0 else 1) * NCH for b in range(B))
    out = nc.dram_tensor("out", [P, nlnc], f32, kind="ExternalOutput")
    nscal = 2 * sum(len(PL[b]) for b in range(B)) + B * C
    scal = nc.dram_tensor("scal", [1, nscal], f32, kind="ExternalOutput")

    stash_bufs = B * NCH if K_DEFER_LN else 2

    with tile.TileContext(nc) as tc:
        with (
            tc.tile_pool(name="xin", bufs=3) as xin,
            tc.tile_pool(name="tin", bufs=3) as tin,
            tc.tile_pool(name="ework", bufs=3) as ework,
            tc.tile_pool(name="qwork", bufs=2) as qwork,
            tc.tile_pool(name="swork", bufs=2) as swork,
            tc.tile_pool(name="misc", bufs=1) as misc,
            tc.tile_pool(name="psum", bufs=1, space="PSUM") as psum,
        ):
            ones = misc.tile([P, 1], bf16)
            nc.vector.memset(ones[:], 1.0)
            accs = misc.tile([P, nlnc], f32)
            nc.vector.memset(accs[:], 0.0)
            junk = misc.tile([P, FCH], f32)
            scal_sb = misc.tile([1, nscal], f32)
            cnt_row = misc.tile([1, B * C], f32)
            pad_bias = {}
            for b in range(B):
                if PAD[b] > 0 and PAD[b] not in pad_bias:
                    pv = misc.tile([P, 1], f32, name=f"pad{b}")
                    nc.vector.memset(pv[:], PAD[b])
                    pad_bias[PAD[b]] = pv

            cnt_ps = [psum.tile([1, C, CNT_FOLD], f32, name=f"cntps{b}")
                      for b in range(B)]
            seg_ps = [psum.tile([1, len(PL[b]), ACC_FOLD], f32, name=f"segps{b}")
                      for b in range(B)]
            int_ps = [psum.tile([1, len(PL[b]), ACC_FOLD], f32, name=f"intps{b}")
                      for b in range(B)]

            # ---- input DMA, in compute order (x then t per chunk)
            x_chs, t_chs = {}, {}
            for b in range(B):
                for ch in range(NCH):
                    sl = slice(ch * FCH, (ch + 1) * FCH)
                    x_ch = xin.tile([P, C, FCH], bf16, tag="x", name=f"x{b}{ch}")
                    nc.sync.dma_start(
                        x_ch[:],
                        x[b * C : (b + 1) * C, :, sl].rearrange("c p f -> p c f"))
                    t_ch = tin.tile([P, C, FCH], bf16, tag="t", name=f"t{b}{ch}")
                    nc.sync.dma_start(
                        t_ch[:],
                        t[b * C : (b + 1) * C, :, sl].rearrange("c p f -> p c f"))
                    x_chs[(b, ch)], t_chs[(b, ch)] = x_ch, t_ch

            def pairsum(slab, ncur, out_ap, tagp):
                """Pairwise-sum ncur leading channels of slab -> out_ap."""
                extras = []
                while ncur > 1 or extras:
                    if ncur == 1:
                        cur_ap = slab[:, 0, :]
                        while len(extras) > 1:
                            nxt = swork.tile([P, FCH], bf16, tag=f"{tagp}x")
                            nc.vector.tensor_tensor(
                                out=nxt[:], in0=cur_ap, in1=extras.pop(), op=OP.add)
                            cur_ap = nxt[:]
                        nc.vector.tensor_tensor(
                            out=out_ap, in0=cur_ap, in1=extras.pop(), op=OP.add)
                        return
                    h = ncur // 2
                    if ncur % 2:
                        extras.append(slab[:, ncur - 1, :])
                    if h == 1 and not extras:
                        nc.vector.tensor_tensor(
                            out=out_ap, in0=slab[:, 0, :], in1=slab[:, 1, :],
                            op=OP.add)
                        return
                    nxt = swork.tile([P, h, FCH], bf16, tag=f"{tagp}{h}")
                    nc.vector.tensor_tensor(
                        out=nxt[:], in0=slab[:, 0:h, :], in1=slab[:, h : 2 * h, :],
                        op=OP.add)
                    slab, ncur = nxt, h

            # ---- main pass, specialized on presence
            last_exp = None
            ln_jobs = []
            col = 0
            for b in range(B):
                pl, al, pad = PL[b], ABS[b], PAD[b]
                cp = len(pl)
                pl_prefix = pl == list(range(cp))
                for ch in range(NCH):
                    x_ch, t_ch = x_chs[(b, ch)], t_chs[(b, ch)]

                    # counts for this (b, chunk) via PE
                    ncb = FCH // CNT_FOLD
                    for fb in range(ncb):
                        nc.tensor.matmul(
                            cnt_ps[b][:], ones[:],
                            t_ch[:, :, fb * CNT_FOLD : (fb + 1) * CNT_FOLD],
                            start=(ch == 0 and fb == 0),
                            stop=(ch == NCH - 1 and fb == ncb - 1))

                    # merged background logit: x0m = x0 + sum_{a in al} x_a
                    if al:
                        acur = x_ch[:, al[0], :]
                        for a in al[1:]:
                            nxt = swork.tile([P, FCH], bf16, tag="bg")
                            nc.vector.tensor_tensor(
                                out=nxt[:], in0=acur, in1=x_ch[:, a, :], op=OP.add)
                            acur = nxt[:]
                        x0m = swork.tile([P, FCH], bf16, tag="bg")
                        nc.vector.tensor_tensor(
                            out=x0m[:], in0=acur, in1=x_ch[:, 0, :], op=OP.add)
                        x0_ap = x0m[:]
                    else:
                        x0_ap = x_ch[:, 0, :]

                    # e_c = exp(m_c), present channels only
                    e_ch = ework.tile([P, cp, FCH], bf16, tag="e", name="e_ch")
                    for i, c in enumerate(pl):
                        src = x0_ap if c == 0 else x_ch[:, c, :]
                        last_exp = nc.scalar.activation(
                            out=e_ch[:, i, :], in_=src, func=FA.Exp)

                    # S (f32), r = 1/S, rb = bf16(r)
                    S = swork.tile([P, FCH], f32, tag="S", bufs=stash_bufs)
                    pairsum(e_ch[:], cp, S[:], "tr")
                    r = swork.tile([P, FCH], f32, tag="r")
                    nc.vector.reciprocal_approx_fast(r[:], S[:])
                    rb = swork.tile([P, 1, FCH], bf16, tag="rb")
                    nc.scalar.activation(out=rb[:, 0, :], in_=r[:], func=FA.Copy)

                    # q = e * r (bf16 wide), seg accum on PE
                    q_ch = qwork.tile([P, cp, FCH], bf16, tag="q", name="q_ch")
                    if K_BCAST:
                        rb_b, e_b = broadcast_tensor_aps(rb[:], e_ch[:])
                        nc.vector.tensor_tensor(
                            out=q_ch[:], in0=e_b, in1=rb_b, op=OP.mult)
                    else:
                        for i in range(cp):
                            nc.vector.tensor_tensor(
                                out=q_ch[:, i, :], in0=e_ch[:, i, :],
                                in1=rb[:, 0, :], op=OP.mult)
                    nfb = FCH // ACC_FOLD
                    for fb in range(nfb):
                        nc.tensor.matmul(
                            seg_ps[b][:], ones[:],
                            q_ch[:, :, fb * ACC_FOLD : (fb + 1) * ACC_FOLD],
                            start=(ch == 0 and fb == 0),
                            stop=(ch == NCH - 1 and fb == nfb - 1))

                    # tq = t * q (reuses the e ring), inter accum on PE
                    tq_ch = ework.tile([P, cp, FCH], bf16, tag="e", name="tq_ch")
                    if pl_prefix:
                        nc.vector.tensor_tensor(
                            out=tq_ch[:], in0=t_ch[:, 0:cp, :], in1=q_ch[:],
                            op=OP.mult)
                    else:
                        for i, c in enumerate(pl):
                            nc.vector.tensor_tensor(
                                out=tq_ch[:, i, :], in0=t_ch[:, c, :],
                                in1=q_ch[:, i, :], op=OP.mult)
                    for fb in range(nfb):
                        nc.tensor.matmul(
                            int_ps[b][:], ones[:],
                            tq_ch[:, :, fb * ACC_FOLD : (fb + 1) * ACC_FOLD],
                            start=(ch == 0 and fb == 0),
                            stop=(ch == NCH - 1 and fb == nfb - 1))

                    # ql = sum_c tq_c (one nonzero per voxel)
                    ql = swork.tile([P, FCH], bf16, tag="ql", bufs=stash_bufs)
                    pairsum(tq_ch[:], cp, ql[:], "tr")

                    # CE pieces
                    jobs = [(ql[:], 0.0, col)]
                    col += 1
                    if pad > 0:
                        jobs += [(S[:], pad_bias[pad][:], col), (S[:], 0.0, col + 1)]
                        col += 2
                    if K_DEFER_LN:
                        ln_jobs += jobs
                    else:
                        for src_ap, bias, cc in jobs:
                            nc.scalar.activation(
                                out=junk[:], in_=src_ap, func=FA.Ln, bias=bias,
                                accum_out=accs[:, cc : cc + 1])

            for src_ap, bias, cc in ln_jobs:
                ln_inst = nc.scalar.activation(
                    out=junk[:], in_=src_ap, func=FA.Ln, bias=bias,
                    accum_out=accs[:, cc : cc + 1])
                add_dep_helper(ln_inst.ins, last_exp.ins, False, "ln after exps")
            assert col == nlnc

            # ---- drains: psum -> scal row (packed by present list)
            off = 0
            for b in range(B):
                cp = len(PL[b])
                nc.vector.tensor_reduce(
                    out=scal_sb[:, off : off + cp], in_=seg_ps[b][:],
                    axis=mybir.AxisListType.X, op=OP.add)
                off += cp
                nc.vector.tensor_reduce(
                    out=scal_sb[:, off : off + cp], in_=int_ps[b][:],
                    axis=mybir.AxisListType.X, op=OP.add)
                off += cp
            for b in range(B):
                nc.vector.tensor_reduce(
                    out=cnt_row[:, b * C : (b + 1) * C], in_=cnt_ps[b][:],
                    axis=mybir.AxisListType.X, op=OP.add)
            nc.vector.tensor_copy(scal_sb[:, off : off + B * C], cnt_row[:])
            off += B * C
            assert off == nscal

            nc.sync.dma_start(out[:], accs[:])
            nc.sync.dma_start(scal[:], scal_sb[:])
    nc.compile()
    return nc


def _get_nc(pres_key):
    if pres_key not in _CACHE:
        _CACHE[pres_key] = _build(pres_key)
    return _CACHE[pres_key]


def _shard_inputs(net_output, target):
    xs = np.ascontiguousarray(net_output).reshape(B, C, NCORES, P, FREE)
    ts = np.ascontiguousarray(target).reshape(B, C, NCORES, P, FREE)
    xmaps, tmaps = [], []
    for k in range(NCORES):
        xk = np.ascontiguousarray(xs[:, :, k]).reshape(B * C, P, FREE)
        tk = np.ascontiguousarray(ts[:, :, k]).reshape(B * C, P, FREE)
        xmaps.append(xk.astype(ml_dtypes.bfloat16))
        tmaps.append(tk.astype(ml_dtypes.bfloat16))  # one-hot: exact in bf16
    return xmaps, tmaps


def _run(nc, in_maps):
    outs = ["out", "scal"]
    if K_SIM:
        import concourse.bass_interp as bass_interp
        sim = bass_interp.MultiCoreSim(nc, NCORES)
        for k in range(NCORES):
            for name, arr in in_maps[k].items():
                sim.cores[k].tensor(name)[:] = arr
        sim.simulate()
        return [{o: sim.cores[k].tensor(o).copy() for o in outs}
                for k in range(NCORES)]
    from concourse.bass_utils import run_bass_kernel_spmd
    return run_bass_kernel_spmd(
        nc, in_maps, core_ids=list(range(NCORES))).results


def _finish(results, pres):
    PL, ABS, PAD = _spec_from_presence(pres)
    cols = []
    for b in range(B):
        for ch in range(NCH):
            cols.append(("ql", b))
            if PAD[b] > 0:
                cols += [("Spad", b), ("S", b)]

    ln = np.zeros(len(cols), dtype=np.float64)
    nscal = 2 * sum(len(PL[b]) for b in range(B)) + B * C
    sc = np.zeros(nscal, dtype=np.float64)
    for r in results:
        ln += r["out"].astype(np.float64).sum(axis=0)
        sc += r["scal"].astype(np.float64).reshape(-1)

    sign = {"ql": -1.0, "Spad": 1.0, "S": -1.0}
    ce = sum(sign[kind] * v for v, (kind, _) in zip(ln, cols)) / NVOX

    seg = np.zeros((B, C)); inter = np.zeros((B, C))
    off = 0
    for b in range(B):
        cp = len(PL[b])
        seg[b, PL[b]] = sc[off : off + cp]; off += cp
        inter[b, PL[b]] = sc[off : off + cp]; off += cp
    cnt = sc[off : off + B * C].reshape(B, C)

    pres_dev = cnt > 0.5
    n = pres_dev.sum(axis=1).astype(np.float64)
    dice_c = 2.0 * inter / (cnt + seg + 1e-5)
    dice_i = 1.0 - (pres_dev * dice_c).sum(axis=1) / n
    dc = dice_i.mean()
    return np.asarray(0.5 * ce + 0.5 * dc, dtype=np.float32), pres_dev


def kernel(net_output, target):
    net_output = np.asarray(net_output)
    target = np.asarray(target)
    # build-time presence scan (device re-derives it; host verifies below)
    pres = target.reshape(B, C, -1).max(axis=2) > 0.5
    for _attempt in range(2):
        pres_key = tuple(bool(v) for v in pres.reshape(-1))
        nc = _get_nc(pres_key)
        xmaps, tmaps = _shard_inputs(net_output, target)
        results = _run(nc, [{"x": xmaps[k], "t": tmaps[k]} for k in range(NCORES)])
        loss, pres_dev = _finish(results, pres)
        if np.array_equal(pres_dev, pres):
            return loss
        pres = pres_dev  # specialize on the true pattern and rerun
    raise RuntimeError("presence pattern did not converge")


# revision 8
# speedup vs baseline: 2.3674x; 1.0188x over previous
"""DC_CE_Marginal_loss for Trainium2 — 8-core data-parallel Bass kernel.

Single fused NEFF per core (D-sharded, pure data parallel). The program is
JIT-specialized on the per-sample present-class pattern (scanned on host at
build time); the device re-derives the label counts every run (PE matmul
pass over the one-hot target) and the host verifies the pattern against the
baked one, rebuilding + rerunning on mismatch — so any input is handled
correctly, and the common path is a single compile per process.

Math (masked re-expression, per sample b with present set Pb, absent set
Ab, pad_b = max |Pb'| - |Pb|):
  m_0   = x_0 + sum_{a in Ab} x_a        (merge_prediction)
  e_c   = exp(m_c) for c in Pb;  S = sum_c e_c;  q_c = e_c / S
  seg_c = sum_v q_c;  inter_c = sum_v t_c q_c;  ql = sum_c t_c q_c
  CE    = mean_v [ln(S+pad) - ln S - ln ql]  (pad=0 -> just -mean ln ql)
  dice from seg/inter/counts as usual (host, tiny).

Engine mapping per (sample, chunk): DVE runs the bf16 trees/products in 2x
mode; ACT runs one wide exp per chunk plus the deferred lns; PE (otherwise
idle) does every per-class reduction as ones-stationary matmuls folding
into PSUM (counts, seg, inter). Chunks are software-pipelined: chunk k+1's
merge+exp are emitted before chunk k's main DVE block so the serial ACT
exp block overlaps DVE work; all lns run after the last exp (one act-table
switch) while DVE finishes the last chunk. Samples are ordered so the one
with fewest present classes is last (shortest tail).
"""
import os
import numpy as np
import ml_dtypes

B, C, D, H, W = 2, 8, 64, 160, 160
NCORES = 8
DS = D // NCORES
PLANE = DS * H * W          # voxels per (b,c) plane per core = 204800
P = 128
FREE = PLANE // P           # 1600
FCH = 800                   # chunk free size
NCH = FREE // FCH           # chunks per sample
NVOX = B * D * H * W

FOLD = 50                   # psum fold width (FCH = 16*50)

K_SIM = os.environ.get("K_SIM", "0") == "1"
K_RB_DVE = os.environ.get("K_RB_DVE", "1") == "1"

_CACHE: dict = {}


def _spec_from_presence(pres):
    """pres: [B, C] bool -> per-sample present/absent lists and pads."""
    pl = [[c for c in range(C) if pres[b, c]] for b in range(B)]
    al = [[c for c in range(C) if not pres[b, c]] for b in range(B)]
    n = [len(p) for p in pl]
    mx = max(n)
    pad = [float(mx - nb) for nb in n]
    assert all(0 in p for p in pl), "background must be present"
    return pl, al, pad


def _sample_order(PL):
    """Fullest samples first: the last chunk (tail) is the cheapest."""
    return sorted(range(B), key=lambda b: -len(PL[b]))


def _scal_offsets(PL):
    """Per-sample (seg, inter) column offsets in the packed scal row."""
    offs, off = {}, 0
    for b in range(B):
        cp = len(PL[b])
        offs[b] = (off, off + cp)
        off += 2 * cp
    return offs


def _build(pres_key):
    import concourse.bacc as bacc
    import concourse.tile as tile
    from concourse import mybir
    from concourse.tile import add_dep_helper
    from concourse.bass import broadcast_tensor_aps
    from concourse.dve_ops import RECIP_APPROX_FAST_CONSTS, RECIPROCAL_APPROX_FAST

    pres = np.array(pres_key, dtype=bool).reshape(B, C)
    PL, ABS, PAD = _spec_from_presence(pres)
    ORD = _sample_order(PL)

    FA = mybir.ActivationFunctionType
    OP = mybir.AluOpType
    f32, bf16 = mybir.dt.float32, mybir.dt.bfloat16

    nc = bacc.Bacc("TRN2", num_devices=NCORES, name="loss_fused")
    x = nc.dram_tensor("x", [B * C, P, FREE], bf16, kind="ExternalInput")
    t = nc.dram_tensor("t", [B * C, P, FREE], bf16, kind="ExternalInput")
    nlnc = sum((3 if PAD[b] > 0 else 1) * NCH for b in range(B))
    out = nc.dram_tensor("out", [P, nlnc], f32, kind="ExternalOutput")
    nscal = 2 * sum(len(PL[b]) for b in range(B)) + B * C
    scal = nc.dram_tensor("scal", [1, nscal], f32, kind="ExternalOutput")

    chunks = [(b, ch) for b in ORD for ch in range(NCH)]

    with tile.TileContext(nc) as tc:
        with (
            tc.tile_pool(name="xin", bufs=3) as xin,
            tc.tile_pool(name="tin", bufs=3) as tin,
            tc.tile_pool(name="ework", bufs=3) as ework,
            tc.tile_pool(name="qwork", bufs=2) as qwork,
            tc.tile_pool(name="swork", bufs=2) as swork,
            tc.tile_pool(name="misc", bufs=1) as misc,
            tc.tile_pool(name="psum", bufs=1, space="PSUM") as psum,
        ):
            # ---- input DMA first so nothing gates the transfers
            x_chs, t_chs = {}, {}
            for b, ch in chunks:
                sl = slice(ch * FCH, (ch + 1) * FCH)
                x_ch = xin.tile([P, C, FCH], bf16, tag="x", name=f"x{b}{ch}")
                nc.sync.dma_start(
                    x_ch[:],
                    x[b * C : (b + 1) * C, :, sl].rearrange("c p f -> p c f"))
                t_ch = tin.tile([P, C, FCH], bf16, tag="t", name=f"t{b}{ch}")
                nc.sync.dma_start(
                    t_ch[:],
                    t[b * C : (b + 1) * C, :, sl].rearrange("c p f -> p c f"))
                x_chs[(b, ch)], t_chs[(b, ch)] = x_ch, t_ch

            ones = misc.tile([P, 1], bf16)
            nc.vector.memset(ones[:], 1.0)
            accs = misc.tile([P, nlnc], f32)
            nc.vector.memset(accs[:], 0.0)
            junk = misc.tile([P, FCH], f32)
            scal_sb = misc.tile([1, nscal], f32)
            pad_bias = {}
            for b in range(B):
                if PAD[b] > 0 and PAD[b] not in pad_bias:
                    pv = misc.tile([P, 1], f32, name=f"pad{b}")
                    nc.vector.memset(pv[:], PAD[b])
                    pad_bias[PAD[b]] = pv

            cnt_ps = [psum.tile([1, C, FOLD], f32, name=f"cntps{b}")
                      for b in range(B)]
            seg_ps = [psum.tile([1, len(PL[b]), FOLD], f32, name=f"segps{b}")
                      for b in range(B)]
            int_ps = [psum.tile([1, len(PL[b]), FOLD], f32, name=f"intps{b}")
                      for b in range(B)]

            NFB = FCH // FOLD

            def pairsum(slab, ncur, out_ap):
                """Pairwise-sum ncur leading channels of slab into out_ap."""
                extras = []
                while True:
                    if ncur == 1:
                        cur_ap = slab[:, 0, :]
                        assert extras
                        while len(extras) > 1:
                            nxt = swork.tile([P, FCH], bf16, tag="trx")
                            nc.vector.tensor_tensor(
                                out=nxt[:], in0=cur_ap, in1=extras.pop(), op=OP.add)
                            cur_ap = nxt[:]
                        nc.vector.tensor_tensor(
                            out=out_ap, in0=cur_ap, in1=extras.pop(), op=OP.add)
                        return
                    h = ncur // 2
                    if ncur % 2:
                        extras.append(slab[:, ncur - 1, :])
                    if h == 1 and not extras:
                        nc.vector.tensor_tensor(
                            out=out_ap, in0=slab[:, 0, :], in1=slab[:, 1, :],
                            op=OP.add)
                        return
                    nxt = swork.tile([P, h, FCH], bf16, tag=f"tr{h}")
                    nc.vector.tensor_tensor(
                        out=nxt[:], in0=slab[:, 0:h, :], in1=slab[:, h : 2 * h, :],
                        op=OP.add)
                    slab, ncur = nxt, h

            e_chs = {}
            last_exp = [None]
            ln_jobs = []
            col = [0]

            def pre(k):
                """Chunk k's mask-free prologue: merge (DVE) + wide exp (ACT)."""
                b, ch = chunks[k]
                pl, al = PL[b], ABS[b]
                cp = len(pl)
                x_ch = x_chs[(b, ch)]
                if al:
                    acur = x_ch[:, al[0], :]
                    for a in al[1:]:
                        nxt = swork.tile([P, FCH], bf16, tag="bg")
                        nc.vector.tensor_tensor(
                            out=nxt[:], in0=acur, in1=x_ch[:, a, :], op=OP.add)
                        acur = nxt[:]
                    nc.vector.tensor_tensor(
                        out=x_ch[:, 0, :], in0=acur, in1=x_ch[:, 0, :], op=OP.add)
                e_ch = ework.tile([P, cp, FCH], bf16, tag="e", name="e_ch")
                if pl == list(range(cp)):
                    last_exp[0] = nc.scalar.activation(
                        out=e_ch[:], in_=x_ch[:, 0:cp, :], func=FA.Exp)
                else:
                    for i, c in enumerate(pl):
                        last_exp[0] = nc.scalar.activation(
                            out=e_ch[:, i, :], in_=x_ch[:, c, :], func=FA.Exp)
                e_chs[k] = e_ch

            def main(k):
                b, ch = chunks[k]
                pl, pad = PL[b], PAD[b]
                cp = len(pl)
                pl_prefix = pl == list(range(cp))
                t_ch = t_chs[(b, ch)]
                e_ch = e_chs.pop(k)

                S = swork.tile([P, FCH], f32, tag="S", bufs=B * NCH)
                pairsum(e_ch[:], cp, S[:])
                rb = swork.tile([P, 1, FCH], bf16, tag="rb")
                if K_RB_DVE:
                    cst = RECIP_APPROX_FAST_CONSTS
                    nc.vector._custom_dve(
                        RECIPROCAL_APPROX_FAST, out=rb[:, 0, :], in0=S[:],
                        s0=cst["s0"], s1=cst["s1"], imm2=cst["imm2"])
                else:
                    r = swork.tile([P, FCH], f32, tag="r")
                    nc.vector.reciprocal_approx_fast(r[:], S[:])
                    nc.scalar.activation(out=rb[:, 0, :], in_=r[:], func=FA.Copy)

                q_ch = qwork.tile([P, cp, FCH], bf16, tag="q", name="q_ch")
                rb_b, e_b = broadcast_tensor_aps(rb[:], e_ch[:])
                nc.vector.tensor_tensor(out=q_ch[:], in0=e_b, in1=rb_b, op=OP.mult)

                # dense PE stream 1: counts + seg (both inputs ready here)
                for fb in range(NFB):
                    nc.tensor.matmul(
                        cnt_ps[b][:], ones[:],
                        t_ch[:, :, fb * FOLD : (fb + 1) * FOLD],
                        start=(ch == 0 and fb == 0),
                        stop=(ch == NCH - 1 and fb == NFB - 1))
                for fb in range(NFB):
                    nc.tensor.matmul(
                        seg_ps[b][:], ones[:],
                        q_ch[:, :, fb * FOLD : (fb + 1) * FOLD],
                        start=(ch == 0 and fb == 0),
                        stop=(ch == NCH - 1 and fb == NFB - 1))

                tq_ch = ework.tile([P, cp, FCH], bf16, tag="e", name="tq_ch")
                if pl_prefix:
                    nc.vector.tensor_tensor(
                        out=tq_ch[:], in0=t_ch[:, 0:cp, :], in1=q_ch[:],
                        op=OP.mult)
                else:
                    for i, c in enumerate(pl):
                        nc.vector.tensor_tensor(
                            out=tq_ch[:, i, :], in0=t_ch[:, c, :],
                            in1=q_ch[:, i, :], op=OP.mult)
                for fb in range(NFB):
                    nc.tensor.matmul(
                        int_ps[b][:], ones[:],
                        tq_ch[:, :, fb * FOLD : (fb + 1) * FOLD],
                        start=(ch == 0 and fb == 0),
                        stop=(ch == NCH - 1 and fb == NFB - 1))

                ql = swork.tile([P, FCH], bf16, tag="ql", bufs=B * NCH)
                pairsum(tq_ch[:], cp, ql[:])

                ln_jobs.append((ql[:], 0.0, col[0]))
                col[0] += 1
                if pad > 0:
                    ln_jobs.append((S[:], pad_bias[pad][:], col[0]))
                    ln_jobs.append((S[:], 0.0, col[0] + 1))
                    col[0] += 2

                if ch == NCH - 1:  # sample finished: drain its psum rows
                    oseg, oint = _scal_offsets(PL)[b]
                    nc.vector.tensor_reduce(
                        out=scal_sb[:, oseg : oseg + cp], in_=seg_ps[b][:],
                        axis=mybir.AxisListType.X, op=OP.add)
                    nc.vector.tensor_reduce(
                        out=scal_sb[:, oint : oint + cp], in_=int_ps[b][:],
                        axis=mybir.AxisListType.X, op=OP.add)
                    ocnt = 2 * sum(len(PL[bb]) for bb in range(B)) + b * C
                    nc.vector.tensor_reduce(
                        out=scal_sb[:, ocnt : ocnt + C], in_=cnt_ps[b][:],
                        axis=mybir.AxisListType.X, op=OP.add)

            # software pipeline: pre(k+1) lands before main(k)
            pre(0)
            for k in range(len(chunks)):
                if k + 1 < len(chunks):
                    pre(k + 1)
                main(k)

            for src_ap, bias, cc in ln_jobs:
                ln_inst = nc.scalar.activation(
                    out=junk[:], in_=src_ap, func=FA.Ln, bias=bias,
                    accum_out=accs[:, cc : cc + 1])
                add_dep_helper(ln_inst.ins, last_exp[0].ins, False, "ln after exps")
            assert col[0] == nlnc

            nc.sync.dma_start(out[:], accs[:])
            nc.sync.dma_start(scal[:], scal_sb[:])
    nc.compile()
    return nc


def _get_nc(pres_key):
    if pres_key not in _CACHE:
        _CACHE[pres_key] = _build(pres_key)
    return _CACHE[pres_key]


def _shard_inputs(net_output, target):
    xs = np.ascontiguousarray(net_output).reshape(B, C, NCORES, P, FREE)
    ts = np.ascontiguousarray(target).reshape(B, C, NCORES, P, FREE)
    xmaps, tmaps = [], []
    for k in range(NCORES):
        xk = np.ascontiguousarray(xs[:, :, k]).reshape(B * C, P, FREE)
        tk = np.ascontiguousarray(ts[:, :, k]).reshape(B * C, P, FREE)
        xmaps.append(xk.astype(ml_dtypes.bfloat16))
        tmaps.append(tk.astype(ml_dtypes.bfloat16))  # one-hot: exact in bf16
    return xmaps, tmaps


def _run(nc, in_maps):
    outs = ["out", "scal"]
    if K_SIM:
        import concourse.bass_interp as bass_interp
        sim = bass_interp.MultiCoreSim(nc, NCORES)
        for k in range(NCORES):
            for name, arr in in_maps[k].items():
                sim.cores[k].tensor(name)[:] = arr
        sim.simulate()
        return [{o: sim.cores[k].tensor(o).copy() for o in outs}
                for k in range(NCORES)]
    from concourse.bass_utils import run_bass_kernel_spmd
    return run_bass_kernel_spmd(
        nc, in_maps, core_ids=list(range(NCORES))).results


def _finish(results, pres):
    PL, ABS, PAD = _spec_from_presence(pres)
    ORD = _sample_order(PL)
    cols = []
    for b in ORD:
        for ch in range(NCH):
            cols.append(("ql", b))
            if PAD[b] > 0:
                cols += [("Spad", b), ("S", b)]

    nscal = 2 * sum(len(PL[b]) for b in range(B)) + B * C
    ln = np.zeros(len(cols), dtype=np.float64)
    sc = np.zeros(nscal, dtype=np.float64)
    for r in results:
        ln += r["out"].astype(np.float64).sum(axis=0)
        sc += r["scal"].astype(np.float64).reshape(-1)

    sign = {"ql": -1.0, "Spad": 1.0, "S": -1.0}
    ce = sum(sign[kind] * v for v, (kind, _) in zip(ln, cols)) / NVOX

    offs = _scal_offsets(PL)
    seg = np.zeros((B, C)); inter = np.zeros((B, C))
    for b in range(B):
        cp = len(PL[b])
        oseg, oint = offs[b]
        seg[b, PL[b]] = sc[oseg : oseg + cp]
        inter[b, PL[b]] = sc[oint : oint + cp]
    ocnt = 2 * sum(len(PL[b]) for b in range(B))
    cnt = sc[ocnt : ocnt + B * C].reshape(B, C)

    pres_dev = cnt > 0.5
    n = pres_dev.sum(axis=1).astype(np.float64)
    dice_c = 2.0 * inter / (cnt + seg + 1e-5)
    dice_i = 1.0 - (pres_dev * dice_c).sum(axis=1) / n
    dc = dice_i.mean()
    return np.asarray(0.5 * ce + 0.5 * dc, dtype=np.float32), pres_dev


def kernel(net_output, target):
    net_output = np.asarray(net_output)
    target = np.asarray(target)
    # build-time presence scan (device re-derives it; host verifies below)
    pres = target.reshape(B, C, -1).max(axis=2) > 0.5
    for _attempt in range(2):
        pres_key = tuple(bool(v) for v in pres.reshape(-1))
        nc = _get_nc(pres_key)
        xmaps, tmaps = _shard_inputs(net_output, target)
        results = _run(nc, [{"x": xmaps[k], "t": tmaps[k]} for k in range(NCORES)])
        loss, pres_dev = _finish(results, pres)
        if np.array_equal(pres_dev, pres):
            return loss
        pres = pres_dev  # specialize on the true pattern and rerun
    raise RuntimeError("presence pattern did not converge")


# revision 16
# speedup vs baseline: 2.5105x; 1.0604x over previous
"""DC_CE_Marginal_loss for Trainium2 — 8-core data-parallel Bass kernel.

Single fused NEFF per core (D-sharded, pure data parallel). The program is
JIT-specialized on the per-sample present-class pattern (scanned on host at
build time); the device re-derives the label counts every run (PE matmul
pass over the one-hot target) and the host verifies the pattern against the
baked one, rebuilding + rerunning on mismatch — so any input is handled
correctly, and the common path is a single compile per process.

Math (masked re-expression, per sample b with present set Pb, absent set
Ab, pad_b = max |Pb'| - |Pb|):
  m_0   = x_0 + sum_{a in Ab} x_a        (merge_prediction)
  e_c   = exp(m_c) for c in Pb;  S = sum_c e_c;  q_c = e_c / S
  seg_c = sum_v q_c;  inter_c = sum_v t_c q_c;  ql = sum_c t_c q_c
  CE    = mean_v [ln(S+pad) - ln S - ln ql]  (pad=0 -> just -mean ln ql)
  dice from seg/inter/counts as usual (host, tiny).

Engine mapping per (sample, chunk): DVE runs the bf16 trees/products in 2x
mode; ACT runs one wide exp per chunk plus the deferred lns; PE (otherwise
idle) does every per-class reduction as ones-stationary matmuls folding
into PSUM (counts, seg, inter). Chunks are software-pipelined: chunk k+1's
merge+exp are emitted before chunk k's main DVE block so the serial ACT
exp block overlaps DVE work; all lns run after the last exp (one act-table
switch) while DVE finishes the last chunk. Samples are ordered so the one
with fewest present classes is last (shortest tail).
"""
import os
import numpy as np
import ml_dtypes

B, C, D, H, W = 2, 8, 64, 160, 160
NCORES = 8
DS = D // NCORES
PLANE = DS * H * W          # voxels per (b,c) plane per core = 204800
P = 128
FREE = PLANE // P           # 1600
FCH = 800                   # chunk free size
NCH = FREE // FCH           # chunks per sample
NVOX = B * D * H * W

FOLD = 50                   # psum fold width (FCH = 16*50)

K_SIM = os.environ.get("K_SIM", "0") == "1"
K_RB_DVE = os.environ.get("K_RB_DVE", "1") == "1"

_CACHE: dict = {}


def _spec_from_presence(pres):
    """pres: [B, C] bool -> per-sample present/absent lists and pads."""
    pl = [[c for c in range(C) if pres[b, c]] for b in range(B)]
    al = [[c for c in range(C) if not pres[b, c]] for b in range(B)]
    n = [len(p) for p in pl]
    mx = max(n)
    pad = [float(mx - nb) for nb in n]
    assert all(0 in p for p in pl), "background must be present"
    return pl, al, pad


def _sample_order(PL):
    """Fullest samples first: the last chunk (tail) is the cheapest."""
    return sorted(range(B), key=lambda b: -len(PL[b]))


def _chunk_order(PL):
    """Ascending present-count: cheap exps open the pipeline, and the
    closing chunk is a pad==0 sample (single ln in the tail)."""
    order = _sample_order(PL)[::-1]
    return [(b, ch) for b in order for ch in range(NCH)]


def _scal_offsets(PL):
    """Per-sample (seg, inter) column offsets in the packed scal row."""
    offs, off = {}, 0
    for b in range(B):
        cp = len(PL[b])
        offs[b] = (off, off + cp)
        off += 2 * cp
    return offs


def _build(pres_key):
    import concourse.bacc as bacc
    import concourse.tile as tile
    from concourse import mybir
    from concourse.tile import add_dep_helper
    from concourse.bass import broadcast_tensor_aps
    from concourse.dve_ops import RECIP_APPROX_FAST_CONSTS, RECIPROCAL_APPROX_FAST

    pres = np.array(pres_key, dtype=bool).reshape(B, C)
    PL, ABS, PAD = _spec_from_presence(pres)
    ORD = _sample_order(PL)

    FA = mybir.ActivationFunctionType
    OP = mybir.AluOpType
    f32, bf16 = mybir.dt.float32, mybir.dt.bfloat16

    nc = bacc.Bacc("TRN2", num_devices=NCORES, name="loss_fused")
    x = nc.dram_tensor("x", [B * C, P, FREE], bf16, kind="ExternalInput")
    t = nc.dram_tensor("t", [B * C, P, FREE], bf16, kind="ExternalInput")
    nlnc = sum((3 if PAD[b] > 0 else 1) * NCH for b in range(B))
    out = nc.dram_tensor("out", [P, nlnc], f32, kind="ExternalOutput")
    nscal = 2 * sum(len(PL[b]) for b in range(B)) + B * C
    scal = nc.dram_tensor("scal", [1, nscal], f32, kind="ExternalOutput")

    chunks = _chunk_order(PL)

    with tile.TileContext(nc) as tc:
        with (
            tc.tile_pool(name="xin", bufs=3) as xin,
            tc.tile_pool(name="tin", bufs=3) as tin,
            tc.tile_pool(name="ework", bufs=2) as ework,
            tc.tile_pool(name="qwork", bufs=2) as qwork,
            tc.tile_pool(name="swork", bufs=2) as swork,
            tc.tile_pool(name="misc", bufs=1) as misc,
            tc.tile_pool(name="psum", bufs=1, space="PSUM") as psum,
        ):
            # ---- input DMA first so nothing gates the transfers
            x_chs, t_chs = {}, {}
            for b, ch in chunks:
                sl = slice(ch * FCH, (ch + 1) * FCH)
                x_ch = xin.tile([P, C, FCH], bf16, tag="x", name=f"x{b}{ch}")
                nc.sync.dma_start(
                    x_ch[:],
                    x[b * C : (b + 1) * C, :, sl].rearrange("c p f -> p c f"))
                t_ch = tin.tile([P, C, FCH], bf16, tag="t", name=f"t{b}{ch}")
                nc.sync.dma_start(
                    t_ch[:],
                    t[b * C : (b + 1) * C, :, sl].rearrange("c p f -> p c f"))
                x_chs[(b, ch)], t_chs[(b, ch)] = x_ch, t_ch

            ones = misc.tile([P, 1], bf16)
            nc.vector.memset(ones[:], 1.0)
            accs = misc.tile([P, nlnc], f32)
            nc.vector.memset(accs[:], 0.0)
            junk = misc.tile([P, FCH], f32)
            scal_sb = misc.tile([1, nscal], f32)
            pad_bias = {}
            for b in range(B):
                if PAD[b] > 0 and PAD[b] not in pad_bias:
                    pv = misc.tile([P, 1], f32, name=f"pad{b}")
                    nc.vector.memset(pv[:], PAD[b])
                    pad_bias[PAD[b]] = pv

            TF2 = FCH // 4                     # t pre-folded twice on gpsimd
            cnt_ps = [psum.tile([1, C, FOLD], f32, name=f"cntps{b}")
                      for b in range(B)]
            seg_ps = [psum.tile([1, len(PL[b]), FOLD], f32, name=f"segps{b}")
                      for b in range(B)]
            int_ps = [psum.tile([1, len(PL[b]), FOLD], f32, name=f"intps{b}")
                      for b in range(B)]

            NFB = FCH // FOLD

            def pairsum(slab, ncur, out_ap):
                """Pairwise-sum ncur leading channels of slab into out_ap."""
                extras = []
                while True:
                    if ncur == 1:
                        cur_ap = slab[:, 0, :]
                        assert extras
                        while len(extras) > 1:
                            nxt = swork.tile([P, FCH], bf16, tag="trx")
                            nc.vector.tensor_tensor(
                                out=nxt[:], in0=cur_ap, in1=extras.pop(), op=OP.add)
                            cur_ap = nxt[:]
                        nc.vector.tensor_tensor(
                            out=out_ap, in0=cur_ap, in1=extras.pop(), op=OP.add)
                        return
                    h = ncur // 2
                    if ncur % 2:
                        extras.append(slab[:, ncur - 1, :])
                    if h == 1 and not extras:
                        nc.vector.tensor_tensor(
                            out=out_ap, in0=slab[:, 0, :], in1=slab[:, 1, :],
                            op=OP.add)
                        return
                    nxt = swork.tile([P, h, FCH], bf16, tag=f"tr{h}")
                    nc.vector.tensor_tensor(
                        out=nxt[:], in0=slab[:, 0:h, :], in1=slab[:, h : 2 * h, :],
                        op=OP.add)
                    slab, ncur = nxt, h

            e_chs = {}
            last_exp = [None]
            ln_jobs = []
            col = [0]

            def pre(k):
                """Chunk k's mask-free prologue: merge + wide exp (ACT).
                The merge tree runs on DVE for the opening chunk (latency
                critical) and on the idle GPSIMD for later ones."""
                b, ch = chunks[k]
                pl, al = PL[b], ABS[b]
                cp = len(pl)
                x_ch = x_chs[(b, ch)]
                if al:
                    meng = nc.vector if k == 0 else nc.gpsimd
                    acur = x_ch[:, al[0], :]
                    for a in al[1:]:
                        nxt = swork.tile([P, FCH], bf16, tag="bg")
                        meng.tensor_tensor(
                            out=nxt[:], in0=acur, in1=x_ch[:, a, :], op=OP.add)
                        acur = nxt[:]
                    meng.tensor_tensor(
                        out=x_ch[:, 0, :], in0=acur, in1=x_ch[:, 0, :], op=OP.add)
                e_ch = ework.tile([P, cp, FCH], bf16, tag="e", name="e_ch")
                if pl == list(range(cp)):
                    if k == 0 and cp > 2:
                        # channels >=1 don't wait on the background merge;
                        # exp them first so the S-tree can start early
                        last_exp[0] = nc.scalar.activation(
                            out=e_ch[:, 1:cp, :], in_=x_ch[:, 1:cp, :],
                            func=FA.Exp)
                        last_exp[0] = nc.scalar.activation(
                            out=e_ch[:, 0, :], in_=x_ch[:, 0, :], func=FA.Exp)
                    else:
                        last_exp[0] = nc.scalar.activation(
                            out=e_ch[:], in_=x_ch[:, 0:cp, :], func=FA.Exp)
                else:
                    for i, c in enumerate(pl):
                        last_exp[0] = nc.scalar.activation(
                            out=e_ch[:, i, :], in_=x_ch[:, c, :], func=FA.Exp)
                e_chs[k] = e_ch

            def main(k):
                b, ch = chunks[k]
                pl, pad = PL[b], PAD[b]
                cp = len(pl)
                pl_prefix = pl == list(range(cp))
                t_ch = t_chs[(b, ch)]
                e_ch = e_chs.pop(k)

                # counts: pre-fold t twice on the idle GPSIMD (4x fewer PE
                # columns), then a short PE stream into the counts psum
                tf1 = swork.tile([P, C, FCH // 2], bf16, tag="tf1")
                nc.gpsimd.tensor_tensor(
                    out=tf1[:], in0=t_ch[:, :, 0 : FCH // 2],
                    in1=t_ch[:, :, FCH // 2 : FCH], op=OP.add)
                tf2 = swork.tile([P, C, TF2], bf16, tag="tf2")
                nc.gpsimd.tensor_tensor(
                    out=tf2[:], in0=tf1[:, :, 0:TF2], in1=tf1[:, :, TF2 : 2 * TF2],
                    op=OP.add)

                S = swork.tile([P, FCH], f32, tag="S")
                if k == 0 and cp > 2:
                    s1n = swork.tile([P, FCH], bf16, tag="s1n")
                    pairsum(e_ch[:, 1:cp, :], cp - 1, s1n[:])
                    nc.vector.tensor_tensor(
                        out=S[:], in0=s1n[:], in1=e_ch[:, 0, :], op=OP.add)
                else:
                    pairsum(e_ch[:], cp, S[:])
                rb = swork.tile([P, 1, FCH], bf16, tag="rb")
                if K_RB_DVE:
                    cst = RECIP_APPROX_FAST_CONSTS
                    nc.vector._custom_dve(
                        RECIPROCAL_APPROX_FAST, out=rb[:, 0, :], in0=S[:],
                        s0=cst["s0"], s1=cst["s1"], imm2=cst["imm2"])
                else:
                    r = swork.tile([P, FCH], f32, tag="r")
                    nc.vector.reciprocal_approx_fast(r[:], S[:])
                    nc.scalar.activation(out=rb[:, 0, :], in_=r[:], func=FA.Copy)

                q_ch = qwork.tile([P, cp, FCH], bf16, tag="q", name="q_ch")
                rb_b, e_b = broadcast_tensor_aps(rb[:], e_ch[:])
                nc.vector.tensor_tensor(out=q_ch[:], in0=e_b, in1=rb_b, op=OP.mult)

                # dense PE streams; PE deliberately lags ~one chunk
                for fb in range(NFB):
                    nc.tensor.matmul(
                        seg_ps[b][:], ones[:],
                        q_ch[:, :, fb * FOLD : (fb + 1) * FOLD],
                        start=(ch == 0 and fb == 0),
                        stop=(ch == NCH - 1 and fb == NFB - 1))
                ncb = TF2 // FOLD
                for fb in range(ncb):
                    nc.tensor.matmul(
                        cnt_ps[b][:], ones[:],
                        tf2[:, :, fb * FOLD : (fb + 1) * FOLD],
                        start=(ch == 0 and fb == 0),
                        stop=(ch == NCH - 1 and fb == ncb - 1))

                # tq overwrites t in place (t's last reader)
                if pl_prefix:
                    nc.vector.tensor_tensor(
                        out=t_ch[:, 0:cp, :], in0=t_ch[:, 0:cp, :], in1=q_ch[:],
                        op=OP.mult)
                else:
                    for i, c in enumerate(pl):
                        nc.vector.tensor_tensor(
                            out=t_ch[:, i, :], in0=t_ch[:, c, :],
                            in1=q_ch[:, i, :], op=OP.mult)
                tq_ch = t_ch
                for fb in range(NFB):
                    nc.tensor.matmul(
                        int_ps[b][:], ones[:],
                        tq_ch[:, 0:cp, fb * FOLD : (fb + 1) * FOLD],
                        start=(ch == 0 and fb == 0),
                        stop=(ch == NCH - 1 and fb == NFB - 1))

                ql = swork.tile([P, FCH], bf16, tag="ql")
                pairsum(tq_ch[:, 0:cp, :], cp, ql[:])

                # per-chunk lns (table flips hide under DVE work)
                nc.scalar.activation(
                    out=junk[:], in_=ql[:], func=FA.Ln,
                    accum_out=accs[:, col[0] : col[0] + 1])
                col[0] += 1
                if pad > 0:
                    nc.scalar.activation(
                        out=junk[:], in_=S[:], func=FA.Ln, bias=pad_bias[pad][:],
                        accum_out=accs[:, col[0] : col[0] + 1])
                    nc.scalar.activation(
                        out=junk[:], in_=S[:], func=FA.Ln,
                        accum_out=accs[:, col[0] + 1 : col[0] + 2])
                    col[0] += 2

                if ch == NCH - 1:  # sample finished: drain its psum rows
                    oseg, oint = _scal_offsets(PL)[b]
                    nc.vector.tensor_reduce(
                        out=scal_sb[:, oseg : oseg + cp], in_=seg_ps[b][:],
                        axis=mybir.AxisListType.X, op=OP.add)
                    nc.vector.tensor_reduce(
                        out=scal_sb[:, oint : oint + cp], in_=int_ps[b][:],
                        axis=mybir.AxisListType.X, op=OP.add)
                    ocnt = 2 * sum(len(PL[bb]) for bb in range(B)) + b * C
                    nc.vector.tensor_reduce(
                        out=scal_sb[:, ocnt : ocnt + C], in_=cnt_ps[b][:],
                        axis=mybir.AxisListType.X, op=OP.add)

            # software pipeline: pre(k+1) lands before main(k)
            pre(0)
            for k in range(len(chunks)):
                if k + 1 < len(chunks):
                    pre(k + 1)
                main(k)

            assert col[0] == nlnc

            nc.sync.dma_start(out[:], accs[:])
            nc.sync.dma_start(scal[:], scal_sb[:])
    nc.compile()
    return nc


def _get_nc(pres_key):
    if pres_key not in _CACHE:
        _CACHE[pres_key] = _build(pres_key)
    return _CACHE[pres_key]


def _shard_inputs(net_output, target):
    xs = np.ascontiguousarray(net_output).reshape(B, C, NCORES, P, FREE)
    ts = np.ascontiguousarray(target).reshape(B, C, NCORES, P, FREE)
    xmaps, tmaps = [], []
    for k in range(NCORES):
        xk = np.ascontiguousarray(xs[:, :, k]).reshape(B * C, P, FREE)
        tk = np.ascontiguousarray(ts[:, :, k]).reshape(B * C, P, FREE)
        xmaps.append(xk.astype(ml_dtypes.bfloat16))
        tmaps.append(tk.astype(ml_dtypes.bfloat16))  # one-hot: exact in bf16
    return xmaps, tmaps


def _run(nc, in_maps):
    outs = ["out", "scal"]
    if K_SIM:
        import concourse.bass_interp as bass_interp
        sim = bass_interp.MultiCoreSim(nc, NCORES)
        for k in range(NCORES):
            for name, arr in in_maps[k].items():
                sim.cores[k].tensor(name)[:] = arr
        sim.simulate()
        return [{o: sim.cores[k].tensor(o).copy() for o in outs}
                for k in range(NCORES)]
    from concourse.bass_utils import run_bass_kernel_spmd
    return run_bass_kernel_spmd(
        nc, in_maps, core_ids=list(range(NCORES))).results


def _finish(results, pres):
    PL, ABS, PAD = _spec_from_presence(pres)
    cols = []
    for b, ch in _chunk_order(PL):
        cols.append(("ql", b))
        if PAD[b] > 0:
            cols += [("Spad", b), ("S", b)]

    nscal = 2 * sum(len(PL[b]) for b in range(B)) + B * C
    ln = np.zeros(len(cols), dtype=np.float64)
    sc = np.zeros(nscal, dtype=np.float64)
    for r in results:
        ln += r["out"].astype(np.float64).sum(axis=0)
        sc += r["scal"].astype(np.float64).reshape(-1)

    sign = {"ql": -1.0, "Spad": 1.0, "S": -1.0}
    ce = sum(sign[kind] * v for v, (kind, _) in zip(ln, cols)) / NVOX

    offs = _scal_offsets(PL)
    seg = np.zeros((B, C)); inter = np.zeros((B, C))
    for b in range(B):
        cp = len(PL[b])
        oseg, oint = offs[b]
        seg[b, PL[b]] = sc[oseg : oseg + cp]
        inter[b, PL[b]] = sc[oint : oint + cp]
    ocnt = 2 * sum(len(PL[b]) for b in range(B))
    cnt = sc[ocnt : ocnt + B * C].reshape(B, C)

    pres_dev = cnt > 0.5
    n = pres_dev.sum(axis=1).astype(np.float64)
    dice_c = 2.0 * inter / (cnt + seg + 1e-5)
    dice_i = 1.0 - (pres_dev * dice_c).sum(axis=1) / n
    dc = dice_i.mean()
    return np.asarray(0.5 * ce + 0.5 * dc, dtype=np.float32), pres_dev


def kernel(net_output, target):
    net_output = np.asarray(net_output)
    target = np.asarray(target)
    # build-time presence scan (device re-derives it; host verifies below)
    pres = target.reshape(B, C, -1).max(axis=2) > 0.5
    for _attempt in range(2):
        pres_key = tuple(bool(v) for v in pres.reshape(-1))
        nc = _get_nc(pres_key)
        xmaps, tmaps = _shard_inputs(net_output, target)
        results = _run(nc, [{"x": xmaps[k], "t": tmaps[k]} for k in range(NCORES)])
        loss, pres_dev = _finish(results, pres)
        if np.array_equal(pres_dev, pres):
            return loss
        pres = pres_dev  # specialize on the true pattern and rerun
    raise RuntimeError("presence pattern did not converge")
